# revision 1
# baseline (speedup 1.0000x reference)
"""AdaptiveGatingMetaNet on 8 Trainium2 NeuronCores (Bass/Tile SPMD).

Data-parallel over batch (1024 rows/core); weights replicated. Per core:
  - meta-net (fp32, transposed layout, batch on the free axis)
  - uncertainty via the Gram trick: unc^2[b,k] = m_b^T diag(w2k) G diag(w2k) m_b
    (G = W1 W1^T), so the [B,K,D] Jacobian is never materialized
  - global max via one 512B AllReduce(max), hidden behind the first (dense)
    combination step
  - gated combination: step 0 dense bf16 (hides collective + index build);
    the ~95%-sparse gating is then exploited by compacting the union of
    active batch columns (sparse_gather/ap_gather) so steps 1..7 run at
    N=512 instead of N=1024; the host scatters results back into the
    untouched feature rows.
"""
import sys
sys.path.insert(0, "/opt/trn_rl_repo")
import numpy as np
from concourse import bass, bacc, tile, mybir

F32 = mybir.dt.float32
F32R = mybir.dt.float32r
BF16 = mybir.dt.bfloat16
I16 = mybir.dt.int16
AX = mybir.AxisListType
ALU = mybir.AluOpType
ACTF = mybir.ActivationFunctionType

D = 1024
H = 256
K = 8
KT_D = D // 128
KT_H = H // 128
N_CORES = 8


def build(BT=1024, debug_outs=False, phase="full", repeat=1, CW=512, upto="full"):
    """Per-core SPMD kernel.

    phase: "full" | "null" (null = passthrough, for dispatch calibration)
    repeat: run the whole computation N times (timing amortization)
    """
    NB = BT // 128
    NS = BT // 512
    nc = bacc.Bacc("TRN2", target_bir_lowering=False, debug=False,
                   num_devices=N_CORES)

    featT_d = nc.dram_tensor("featT", [128, KT_D, BT], F32, kind="ExternalInput")
    w1t_d = nc.dram_tensor("w1t", [128, KT_D, H], F32, kind="ExternalInput")
    w2t_d = nc.dram_tensor("w2t", [128, KT_H, K], F32, kind="ExternalInput")
    w2rows_d = nc.dram_tensor("w2rows", [K, H], F32, kind="ExternalInput")
    b1t_d = nc.dram_tensor("b1t", [128, KT_H], F32, kind="ExternalInput")
    b2row_d = nc.dram_tensor("b2row", [1, K], F32, kind="ExternalInput")
    selmat_d = nc.dram_tensor("selmat", [K, D], F32, kind="ExternalInput")
    ident_d = nc.dram_tensor("ident", [128, 128], F32, kind="ExternalInput")
    scal_d = nc.dram_tensor("scal", [1, 4], F32, kind="ExternalInput")
    mats_d = nc.dram_tensor("mats", [K, 128, KT_D, D], BF16, kind="ExternalInput")
    iota16_d = nc.dram_tensor("iota16", [16, BT // 16], F32, kind="ExternalInput")
    positer_d = nc.dram_tensor("positer", [16, CW // 16], F32, kind="ExternalInput")
    outc_d = nc.dram_tensor("outc", [128, KT_D, CW], F32, kind="ExternalOutput")
    idxout_d = nc.dram_tensor("idxout", [16, CW // 16], F32, kind="ExternalOutput")
    if debug_outs:
        dbg_gated_d = nc.dram_tensor("dbg_gated", [128, NB, K], F32, kind="ExternalOutput")
        dbg_u2_d = nc.dram_tensor("dbg_u2", [128, NB, K], F32, kind="ExternalOutput")

    cc_in = nc.dram_tensor("cc_in", [128], F32)
    cc_out = nc.dram_tensor("cc_out", [128], F32, addr_space="Shared")

    with tile.TileContext(nc) as tc:
        with (
            tc.tile_pool(name="persist", bufs=1) as pp,
            tc.tile_pool(name="work", bufs=2) as wp,
            tc.tile_pool(name="rmats", bufs=2) as mp,
            tc.tile_pool(name="psP", bufs=1, space="PSUM") as psP,
        ):
            # ----- constants (once) -----
            w1t = pp.tile([128, KT_D, H], F32, tag="w1t")
            nc.sync.dma_start(w1t[:], w1t_d[:])
            w2t = pp.tile([128, KT_H, K], F32, tag="w2t")
            nc.sync.dma_start(w2t[:], w2t_d[:])
            b1t = pp.tile([128, KT_H], F32, tag="b1t")
            nc.sync.dma_start(b1t[:], b1t_d[:])
            b2r = pp.tile([1, K], F32, tag="b2r")
            nc.sync.dma_start(b2r[:], b2row_d[:])
            sel = pp.tile([K, D], F32, tag="sel")
            nc.sync.dma_start(sel[:], selmat_d[:])
            ident = pp.tile([128, 128], F32, tag="ident")
            nc.sync.dma_start(ident[:], ident_d[:])
            scal = pp.tile([1, 4], F32, tag="scal")
            nc.sync.dma_start(scal[:], scal_d[:])
            w2bc = []
            for k in range(K):
                t = pp.tile([1, H], F32, tag=f"w2row{k}", name=f"w2row{k}")
                nc.sync.dma_start(t[:], w2rows_d[k:k + 1, :])
                tb = pp.tile([128, H], F32, tag=f"w2bc{k}", name=f"w2bc{k}")
                nc.gpsimd.partition_broadcast(tb[:], t[:])
                w2bc.append(tb)
            b2bc = pp.tile([128, K], F32, tag="b2bc")
            nc.gpsimd.partition_broadcast(b2bc[:], b2r[:])

            for rep in range(repeat):
                # ----- x^T load (per repeat re-init) -----
                xT = []
                for kt in range(KT_D):
                    t = pp.tile([128, BT], F32, tag=f"xT{kt}", name=f"xT{kt}_{rep}")
                    nc.sync.dma_start(t[:], featT_d[:, kt, :])
                    xT.append(t)

                if phase == "null":
                    break

                matj = []
                for j in range(K):
                    t = mp.tile([128, KT_D, D], BF16, tag="matj", name=f"matj{j}_{rep}")
                    nc.sync.dma_start(t[:], mats_d[j])
                    matj.append(t)

                # ----- G = W1 @ W1^T -----
                G = pp.tile([128, KT_H, H], F32, tag="G", name=f"G_{rep}")
                for ms in range(KT_H):
                    ps = psP.tile([128, H], F32, tag="small", bufs=2, name=f"gps{ms}_{rep}")
                    for kt in range(KT_D):
                        nc.tensor.matmul(ps[:], w1t[:, kt, ms * 128:(ms + 1) * 128],
                                         w1t[:, kt, :], start=(kt == 0), stop=(kt == KT_D - 1))
                    nc.vector.tensor_copy(G[:, ms, :], ps[:])

                # ----- relu = relu(W1 @ x^T + b1); mask = (relu > 0) == (h > 0) -----
                maskT = pp.tile([128, KT_H, BT], F32, tag="maskT", name=f"maskT_{rep}")
                relu = pp.tile([128, KT_H, BT], F32, tag="relu", name=f"relu_{rep}")
                for ms in range(KT_H):
                    for ns in range(NS):
                        nsl = slice(ns * 512, (ns + 1) * 512)
                        ps = psP.tile([128, 512], F32, tag="A", bufs=3,
                                      name=f"hps{ms}_{ns}_{rep}")
                        for kt in range(KT_D):
                            nc.tensor.matmul(ps[:], w1t[:, kt, ms * 128:(ms + 1) * 128],
                                             xT[kt][:, nsl],
                                             start=(kt == 0), stop=(kt == KT_D - 1))
                        nc.scalar.activation(relu[:, ms, nsl], ps[:],
                                             ACTF.Relu, bias=b1t[:, ms:ms + 1], scale=1.0)
                        nc.vector.tensor_scalar(maskT[:, ms, nsl], relu[:, ms, nsl],
                                                0.0, None, ALU.is_gt)

                # maskB via PE transposes
                maskB = pp.tile([128, NB, H], BF16, tag="maskB", name=f"maskB_{rep}")
                for kt in range(KT_H):
                    for bt in range(NB):
                        tps = psP.tile([128, 128], F32, tag="small", bufs=2,
                                       name=f"tps{kt}_{bt}_{rep}")
                        nc.tensor.transpose(tps[:], maskT[:, kt, bt * 128:(bt + 1) * 128],
                                            ident[:])
                        nc.vector.tensor_copy(maskB[:, bt, kt * 128:(kt + 1) * 128], tps[:])

                # ----- G2 pairs: diag(w2k) G diag(w2k) -----
                G2p = []
                for p in range(K // 2):
                    g2 = pp.tile([128, KT_H, 2 * H], F32, tag=f"g2_{p}", name=f"g2_{p}_{rep}")
                    for half, k in ((0, 2 * p), (1, 2 * p + 1)):
                        hs = slice(half * H, (half + 1) * H)
                        for kt in range(KT_H):
                            nc.vector.tensor_scalar(g2[:, kt, hs], G[:, kt, :],
                                                    w2t[:, kt, k:k + 1], None, ALU.mult)
                            nc.vector.tensor_tensor(g2[:, kt, hs], g2[:, kt, hs],
                                                    w2bc[k][:], ALU.mult)
                    G2p.append(g2)

                # ----- coeffs_B, u2_B -----
                coeffsB = pp.tile([128, NB, K], F32, tag="coeffsB", name=f"coeffsB_{rep}")
                u2 = pp.tile([128, NB, K], F32, tag="u2", name=f"u2_{rep}")
                for bt in range(NB):
                    bsl = slice(bt * 128, (bt + 1) * 128)
                    cps = psP.tile([128, K], F32, tag="small", bufs=2, name=f"cps{bt}_{rep}")
                    for kt in range(KT_H):
                        nc.tensor.matmul(cps[:], relu[:, kt, bsl], w2t[:, kt, :],
                                         start=(kt == 0), stop=(kt == KT_H - 1))
                    nc.vector.tensor_tensor(coeffsB[:, bt, :], cps[:], b2bc[:], ALU.add)
                    for pg in range(2):
                        xk_a = psP.tile([128, 2 * H], F32, tag="xkp0", bufs=1,
                                        name=f"xka_{bt}_{pg}_{rep}")
                        xk_b = psP.tile([128, 2 * H], F32, tag="xkp1", bufs=1,
                                        name=f"xkb_{bt}_{pg}_{rep}")
                        for kt in range(KT_H):
                            nc.tensor.matmul(xk_a[:], maskT[:, kt, bsl],
                                             G2p[2 * pg][:, kt, :],
                                             start=(kt == 0), stop=(kt == KT_H - 1))
                            nc.tensor.matmul(xk_b[:], maskT[:, kt, bsl],
                                             G2p[2 * pg + 1][:, kt, :],
                                             start=(kt == 0), stop=(kt == KT_H - 1))
                        for half_t, xk in ((0, xk_a), (1, xk_b)):
                            for half in range(2):
                                k = 4 * pg + 2 * half_t + half
                                scr = wp.tile([128, H], F32, tag="ttr_scratch",
                                              name=f"scr{bt}_{pg}_{half_t}_{half}_{rep}")
                                nc.vector.tensor_tensor(
                                    scr[:], xk[:, half * H:(half + 1) * H],
                                    maskB[:, bt, :], ALU.mult)
                                nc.vector.tensor_reduce(
                                    u2[:, bt, k:k + 1], scr[:], AX.X, ALU.add)

                # ----- AllReduce max -----
                lmax = wp.tile([128, 1], F32, tag="lmax", name=f"lmax_{rep}")
                nc.vector.tensor_reduce(lmax[:], u2[:], AX.XY, ALU.max)
                nc.gpsimd.dma_start(cc_in[:], lmax[:])
                nc.gpsimd.collective_compute(
                    "AllReduce", ALU.max, replica_groups=[list(range(N_CORES))],
                    ins=[cc_in[:]], outs=[cc_out[:]])
                gmax_col = wp.tile([1, 128], F32, tag="gmax_col", name=f"gmax_{rep}")
                nc.gpsimd.dma_start(gmax_col[:], cc_out[:])
                m2 = wp.tile([1, 1], F32, tag="m2", name=f"m2_{rep}")
                nc.vector.tensor_reduce(m2[:], gmax_col[:], AX.X, ALU.max)
                sq = wp.tile([1, 1], F32, tag="sqm2", name=f"sq_{rep}")
                nc.scalar.activation(sq[:], m2[:], ACTF.Sqrt)
                rs = wp.tile([1, 1], F32, tag="rsq", name=f"rs_{rep}")
                nc.vector.reciprocal(rs[:], sq[:])
                s11 = wp.tile([1, 1], F32, tag="s11", name=f"s11_{rep}")
                nc.vector.tensor_tensor(s11[:], rs[:], scal[:, 1:2], ALU.mult)
                sbc = wp.tile([128, 1], F32, tag="sbc", name=f"sbc_{rep}")
                nc.gpsimd.partition_broadcast(sbc[:], s11[:])
                basebc = wp.tile([128, 1], F32, tag="basebc", name=f"basebc_{rep}")
                nc.gpsimd.partition_broadcast(basebc[:], scal[:, 0:1])

                # ----- thresholds + gating -----
                unionK = pp.tile([128, NB], F32, tag="unionK", name=f"unionK_{rep}")
                gatedB = pp.tile([128, NB, K], F32, tag="gatedB", name=f"gatedB_{rep}")
                for bt in range(NB):
                    sqr = wp.tile([128, K], F32, tag="sqr", name=f"sqr{bt}_{rep}")
                    nc.scalar.activation(sqr[:], u2[:, bt, :], ACTF.Sqrt)
                    th = wp.tile([128, K], F32, tag="th", name=f"th{bt}_{rep}")
                    nc.scalar.activation(th[:], sqr[:], ACTF.Identity,
                                         bias=basebc[:], scale=sbc[:])
                    absc = wp.tile([128, K], F32, tag="absc", name=f"absc{bt}_{rep}")
                    nc.scalar.activation(absc[:], coeffsB[:, bt, :], ACTF.Abs)
                    keep = wp.tile([128, K], F32, tag="keep", name=f"keep{bt}_{rep}")
                    nc.vector.tensor_tensor(keep[:], absc[:], th[:], ALU.is_ge)
                    nc.vector.tensor_tensor(gatedB[:, bt, :], coeffsB[:, bt, :],
                                            keep[:], ALU.mult)
                    nc.vector.tensor_reduce(unionK[:, bt:bt + 1], keep[:], AX.X, ALU.max)
                if debug_outs and rep == repeat - 1:
                    nc.sync.dma_start(dbg_gated_d[:], gatedB[:])
                    nc.sync.dma_start(dbg_u2_d[:], u2[:])

                gatedT = pp.tile([K, BT], F32, tag="gatedT", name=f"gatedT_{rep}")
                for bt in range(NB):
                    tps = psP.tile([K, 128], F32, tag="small", bufs=2,
                                   name=f"tpsg{bt}_{rep}")
                    nc.tensor.transpose(tps[:], gatedB[:, bt, :], ident[:])
                    nc.vector.tensor_copy(gatedT[:, bt * 128:(bt + 1) * 128], tps[:])

                if upto == "gating":
                    continue
                # ----- combination loop -----
                # j=0 runs DENSE (hides the collective + index build);
                # j=1..7 run on the compacted union of active columns (width CW).
                WF = CW // 16
                rcur = wp.tile([128, KT_D, BT], BF16, tag="rmoving", bufs=1, name=f"r0_{rep}")
                for kt in range(KT_D):
                    nc.vector.tensor_copy(rcur[:, kt, :], xT[kt][:])

                # j=0 dense pass
                gb0 = wp.tile([128, BT], F32, tag="gbsb", bufs=1, name=f"gb0_{rep}")
                for ns in range(NS):
                    gps_ = psP.tile([128, 512], F32, tag="small", bufs=2,
                                    name=f"gbp0_{ns}_{rep}")
                    nc.tensor.matmul(gps_[:], sel[:, 0:128],
                                     gatedT[:, ns * 512:(ns + 1) * 512],
                                     start=True, stop=True)
                    nc.scalar.copy(gb0[:, ns * 512:(ns + 1) * 512], gps_[:])
                for m in range(KT_D):
                    for ns in range(NS):
                        nsl = slice(ns * 512, (ns + 1) * 512)
                        ps = psP.tile([128, 512], F32, tag="A", bufs=3,
                                      name=f"lps0_{m}_{ns}_{rep}")
                        for kt in range(KT_D):
                            nc.tensor.matmul(
                                ps[:], matj[0][:, kt, m * 128:(m + 1) * 128],
                                rcur[:, kt, nsl],
                                start=(kt == 0), stop=(kt == KT_D - 1))
                        tmp = wp.tile([128, 512], F32, tag="tmp0", bufs=1,
                                      name=f"tmp{m}_{ns}_{rep}")
                        nc.vector.tensor_tensor(tmp[:], ps[:], gb0[:, nsl], ALU.mult)
                        nc.vector.tensor_tensor(xT[m][:, nsl], xT[m][:, nsl],
                                                tmp[:], ALU.add)

                # --- index machinery (overlaps the j=0 matmuls above) ---
                iota16 = pp.tile([16, BT // 16], F32, tag="iota16", name=f"iota16_{rep}")
                nc.scalar.dma_start(iota16[:], iota16_d[:])
                positer = pp.tile([16, WF], F32, tag="positer", name=f"positer_{rep}")
                nc.scalar.dma_start(positer[:], positer_d[:])
                u16 = wp.tile([16, NB, 8], F32, tag="u16", name=f"u16_{rep}")
                for g in range(8):
                    nc.scalar.dma_start(u16[:, :, g:g + 1],
                                        unionK[g * 16:(g + 1) * 16, :])
                cand = wp.tile([16, BT // 16], F32, tag="cand", name=f"cand_{rep}")
                # cand = iota*u - (1-u):  u in {0,1}
                u16f = u16[:].rearrange("p g b -> p (b g)") if False else None
                # u16 layout: [p16, g, bt] maps to b = bt*128 + g*16 + p16;
                # iota wrapped layout needs b = f*16 + p16 with f = bt*8 + g.
                # Reorder free dims via an AP view: (g, bt) -> (bt, g)
                u16flat = u16[:].rearrange("p b g -> p (b g)")
                nc.vector.tensor_tensor(cand[:], iota16[:], u16flat, ALU.mult)
                um1 = wp.tile([16, BT // 16], F32, tag="um1", name=f"um1_{rep}")
                nc.vector.tensor_scalar(um1[:], u16flat, 1.0, None, ALU.subtract)
                nc.vector.tensor_tensor(cand[:], cand[:], um1[:], ALU.add)
                idxf = wp.tile([16, WF], F32, tag="idxf", name=f"idxf_{rep}")
                cnt = wp.tile([1, 1], mybir.dt.uint32, tag="cnt", name=f"cnt_{rep}")
                nc.gpsimd.sparse_gather(idxf[:], cand[:], num_found=cnt[:])
                # mask junk tail: idx = idx if pos < cnt else -1
                cntf = wp.tile([1, 1], F32, tag="cntf", name=f"cntf_{rep}")
                nc.vector.tensor_copy(cntf[:], cnt[:])
                cntbc = wp.tile([16, 1], F32, tag="cntbc", name=f"cntbc_{rep}")
                nc.gpsimd.partition_broadcast(cntbc[:], cntf[:])
                posok = wp.tile([16, WF], F32, tag="posok", name=f"posok_{rep}")
                nc.vector.tensor_scalar(posok[:], positer[:], cntbc[:], None, ALU.is_lt)
                nc.vector.tensor_tensor(idxf[:], idxf[:], posok[:], ALU.mult)
                pm1 = wp.tile([16, WF], F32, tag="pm1", name=f"pm1_{rep}")
                nc.vector.tensor_scalar(pm1[:], posok[:], 1.0, None, ALU.subtract)
                nc.vector.tensor_tensor(idxf[:], idxf[:], pm1[:], ALU.add)
                nc.scalar.dma_start(idxout_d[:], idxf[:])
                idxf0 = wp.tile([16, WF], F32, tag="idxf0", name=f"idxf0_{rep}")
                nc.vector.tensor_scalar(idxf0[:], idxf[:], 0.0, None, ALU.max)
                idx16 = wp.tile([16, WF], I16, tag="idx16", name=f"idx16_{rep}")
                nc.vector.tensor_copy(idx16[:], idxf0[:])
                idxrep = wp.tile([128, WF], I16, tag="idxrep", name=f"idxrep_{rep}")
                for g in range(8):
                    nc.scalar.dma_start(idxrep[g * 16:(g + 1) * 16, :], idx16[:])

                # gatedTc: compact gated rows
                gatedT16 = pp.tile([16, BT], F32, tag="gatedT16", name=f"gatedT16_{rep}")
                nc.vector.memset(gatedT16[:], 0.0)
                nc.vector.tensor_copy(gatedT16[0:K, :], gatedT[:])
                gatedTc = pp.tile([16, CW], F32, tag="gatedTc", name=f"gatedTc_{rep}")
                nc.gpsimd.ap_gather(gatedTc[:], gatedT16[:], idx16[:], channels=16,
                                    num_elems=BT, d=1, num_idxs=CW)

                # gather compacted x (after j=0 updates)
                xc = []
                for m in range(KT_D):
                    t = pp.tile([128, CW], F32, tag=f"xc{m}", name=f"xc{m}_{rep}")
                    nc.gpsimd.ap_gather(t[:], xT[m][:], idxrep[:], channels=128,
                                        num_elems=BT, d=1, num_idxs=CW)
                    xc.append(t)

                # compact loop j=1..7 (software-pipelined: rc/gbc for j+1
                # are produced during step j right after each xc[m] update)
                def make_gbc(j):
                    gbc = wp.tile([128, CW], F32, tag="gbc", name=f"gbc{j}_{rep}")
                    gcp = psP.tile([128, CW], F32, tag="small", bufs=2,
                                   name=f"gcp{j}_{rep}")
                    nc.tensor.matmul(gcp[:], sel[:, j * 128:(j + 1) * 128],
                                     gatedTc[0:K, :], start=True, stop=True)
                    nc.scalar.copy(gbc[:], gcp[:])
                    return gbc

                if upto == "full":
                    gbc = make_gbc(1)
                    rc = wp.tile([128, KT_D, CW], BF16, tag="rc", name=f"rc1_{rep}")
                    for m in range(KT_D):
                        nc.vector.tensor_tensor(rc[:, m, :], xc[m][:], gbc[:], ALU.mult)
                    for j in range(1, K):
                        gbcn = make_gbc(j + 1) if j + 1 < K else None
                        rcn = (wp.tile([128, KT_D, CW], BF16, tag="rc",
                                       name=f"rc{j+1}_{rep}") if j + 1 < K else None)
                        for m in range(KT_D):
                            ps = psP.tile([128, CW], F32, tag="A", bufs=3,
                                          name=f"clps{j}_{m}_{rep}")
                            for kt in range(KT_D):
                                nc.tensor.matmul(
                                    ps[:], matj[j][:, kt, m * 128:(m + 1) * 128],
                                    rc[:, kt, :],
                                    start=(kt == 0), stop=(kt == KT_D - 1))
                            nc.vector.tensor_tensor(xc[m][:], xc[m][:], ps[:], ALU.add)
                            if rcn is not None:
                                nc.vector.tensor_tensor(rcn[:, m, :], xc[m][:],
                                                        gbcn[:], ALU.mult)
                        rc = rcn

            for kt in range(KT_D):
                if phase != "null" and upto != "gating":
                    nc.sync.dma_start(outc_d[:, kt, :], xc[kt][:])

    nc.finalize()
    return nc


def make_inputs(features_shard_T, W1, b1, W2, b2, task_mats, base_threshold, beta):
    import ml_dtypes
    BT = features_shard_T.shape[1]
    featT = np.ascontiguousarray(
        features_shard_T.reshape(KT_D, 128, BT).transpose(1, 0, 2))
    w1t = np.ascontiguousarray(W1.T.reshape(KT_D, 128, H).transpose(1, 0, 2))
    w2t = np.ascontiguousarray(W2.T.reshape(KT_H, 128, K).transpose(1, 0, 2))
    b1t = np.ascontiguousarray(b1.reshape(KT_H, 128).T)
    sel = np.zeros((K, D), np.float32)
    for j in range(K):
        sel[j, j * 128:(j + 1) * 128] = 1.0
    base_sp = np.log1p(np.exp(np.float32(base_threshold[0]))).astype(np.float32)
    rbeta = np.maximum(np.float32(beta[0]), 0).astype(np.float32)
    scal = np.array([[base_sp, base_sp * rbeta, 0, 0]], np.float32)
    mats = np.ascontiguousarray(
        task_mats.reshape(K, KT_D, 128, D).transpose(0, 2, 1, 3)).astype(ml_dtypes.bfloat16)
    CW = 512
    iota16 = (np.arange(BT).reshape(BT // 16, 16).T).astype(np.float32)
    positer = (np.arange(CW).reshape(CW // 16, 16).T).astype(np.float32)
    return {
        "iota16": np.ascontiguousarray(iota16),
        "positer": np.ascontiguousarray(positer),
        "featT": featT.astype(np.float32),
        "w1t": w1t.astype(np.float32),
        "w2t": w2t.astype(np.float32),
        "w2rows": np.ascontiguousarray(W2).astype(np.float32),
        "b1t": b1t.astype(np.float32),
        "b2row": b2.reshape(1, K).astype(np.float32),
        "selmat": sel,
        "ident": np.eye(128, dtype=np.float32),
        "scal": scal,
        "mats": mats,
    }




_CACHE = {}


def _host_reference(features, W1, b1, W2, b2, task_mats, base_threshold, beta):
    """Pure-numpy fallback (only for pathological gating densities)."""
    f = features.astype(np.float64)
    h = f @ W1.T.astype(np.float64) + b1.astype(np.float64)
    relu_h = np.maximum(h, 0.0)
    coeffs = relu_h @ W2.T.astype(np.float64) + b2.astype(np.float64)
    mask = (h > 0).astype(np.float64)
    G = W1.astype(np.float64) @ W1.T.astype(np.float64)
    u2 = np.zeros((f.shape[0], K))
    for k in range(K):
        V = mask * W2[k].astype(np.float64)[None, :]
        u2[:, k] = np.sum((V @ G) * V, axis=1)
    unc = np.sqrt(u2)
    mx = unc.max()
    if mx > 0:
        unc = unc / mx
    base = np.log1p(np.exp(np.float64(base_threshold.reshape(-1)[0])))
    th = base * (1.0 + max(float(beta.reshape(-1)[0]), 0.0) * unc)
    gated = np.where(np.abs(coeffs) < th, 0.0, coeffs)
    x = f.copy()
    for j in range(K):
        x = x + gated[:, j:j + 1] * (x @ task_mats[j].astype(np.float64))
    return x.astype(np.float32)


def _union_counts(features, W1, b1, W2, b2, base_threshold, beta):
    """Approximate per-core union-active counts (sizing check only)."""
    f = features.astype(np.float32)
    h = f @ W1.T + b1
    coeffs = np.maximum(h, 0) @ W2.T + b2
    mask = (h > 0).astype(np.float32)
    G = (W1 @ W1.T).astype(np.float32)
    u2 = np.zeros((f.shape[0], K), np.float32)
    for k in range(K):
        V = mask * W2[k][None, :]
        u2[:, k] = np.sum((V @ G) * V, axis=1)
    unc = np.sqrt(u2)
    mx = unc.max()
    if mx > 0:
        unc = unc / mx
    base = np.log1p(np.exp(np.float32(base_threshold.reshape(-1)[0])))
    th = base * (1 + max(float(beta.reshape(-1)[0]), 0.0) * unc)
    active = (np.abs(coeffs) >= th).any(axis=1)
    BT = f.shape[0] // N_CORES
    return [int(active[c * BT:(c + 1) * BT].sum()) for c in range(N_CORES)]


def kernel(features, W1, b1, W2, b2, task_mats, proj_W, base_threshold, beta,
           **_unused):
    from concourse.bass_utils import run_bass_kernel_spmd

    features = np.asarray(features, dtype=np.float32)
    W1 = np.asarray(W1, np.float32)
    b1 = np.asarray(b1, np.float32)
    W2 = np.asarray(W2, np.float32)
    b2 = np.asarray(b2, np.float32)
    task_mats = np.asarray(task_mats, np.float32)
    base_threshold = np.asarray(base_threshold, np.float32)
    beta = np.asarray(beta, np.float32)
    B = features.shape[0]
    BT = B // N_CORES
    CW = 512

    # sizing check: the compact loop holds up to CW active columns per core.
    # device/host gate decisions can differ by a couple of borderline samples,
    # so require a safety margin; otherwise fall back to exact host compute.
    counts = _union_counts(features, W1, b1, W2, b2, base_threshold, beta)
    x = None
    if max(counts) <= CW - 8 and B % N_CORES == 0:
        try:
            key = ("nc", BT, CW)
            if key not in _CACHE:
                _CACHE[key] = build(BT=BT, CW=CW)
            nc = _CACHE[key]
            in_maps = []
            for c in range(N_CORES):
                shard_T = np.ascontiguousarray(features[c * BT:(c + 1) * BT].T)
                in_maps.append(make_inputs(shard_T, W1, b1, W2, b2, task_mats,
                                           base_threshold, beta))
            res = run_bass_kernel_spmd(nc, in_maps, core_ids=list(range(N_CORES)))
            outs = []
            for c in range(N_CORES):
                r = res.results[c]
                idx1d = r["idxout"].T.reshape(-1)      # unwrap i = f*16 + p
                valid = idx1d >= 0
                idxs = idx1d[valid].astype(np.int64)
                xcT = r["outc"].transpose(1, 0, 2).reshape(D, CW)
                xfull = features[c * BT:(c + 1) * BT].copy()
                xfull[idxs, :] = xcT[:, valid].T
                outs.append(xfull)
            x = np.concatenate(outs, axis=0)
        except Exception as e:
            import traceback
            print(f"kernel: device path failed ({e!r}); falling back to host",
                  file=sys.stderr)
            traceback.print_exc()
            x = None
    if x is None:
        x = _host_reference(features, W1, b1, W2, b2, task_mats,
                            base_threshold, beta)

    proj_W = np.asarray(proj_W, np.float32)
    if not (proj_W.shape == (D, D) and
            np.array_equal(proj_W, np.eye(D, dtype=proj_W.dtype))):
        x = (x @ proj_W.T).astype(np.float32)
    return np.ascontiguousarray(x.astype(np.float32))



# revision 8
# speedup vs baseline: 1.3073x; 1.3073x over previous
"""AdaptiveGatingMetaNet on 8 Trainium2 NeuronCores (Bass/Tile SPMD).

Data-parallel over batch (1024 rows/core); weights replicated. Per core:
  - meta-net h in float32r (1 cyc/row PE, ~1.5e-4 rel err — gate-flip safe),
    coeffs in fp32
  - uncertainty via the Gram trick in bf16 (host-verified: zero gate flips):
    unc^2[b,k] = m_b^T diag(w2k) G diag(w2k) m_b with G = W1 W1^T; stage-2
    mask-dot fused into single tensor_tensor_reduce ops on DVE
  - global max via one 512B AllReduce(max)
  - combination: step 0 dense bf16 with the gate applied to the moving
    operand (column scale commutes through the contraction); steps 1..7 on
    the compacted union of active batch columns (CW=448, measured max 389)
  - task matrices (16MB bf16) are streamed per repeat across three DMA
    issuers (scalar/gpsimd/sync) with a 3-deep buffer ring for overlap
"""
import sys
sys.path.insert(0, "/opt/trn_rl_repo")
import numpy as np
from concourse import bass, bacc, tile, mybir

F32 = mybir.dt.float32
F32R = mybir.dt.float32r
BF16 = mybir.dt.bfloat16
I16 = mybir.dt.int16
AX = mybir.AxisListType
ALU = mybir.AluOpType
ACTF = mybir.ActivationFunctionType

D = 1024
H = 256
K = 8
KT_D = D // 128
KT_H = H // 128
N_CORES = 8
CW_DEFAULT = 448


def build(BT=1024, debug_outs=False, phase="full", repeat=1, CW=CW_DEFAULT,
          upto="full"):
    """Per-core SPMD kernel.

    phase: "full" | "null" (null = passthrough, for dispatch calibration)
    repeat: run the whole computation N times (timing amortization)
    """
    NB = BT // 128
    NS = BT // 512
    nc = bacc.Bacc("TRN2", target_bir_lowering=False, debug=False,
                   num_devices=N_CORES)

    featT_d = nc.dram_tensor("featT", [128, KT_D, BT], F32, kind="ExternalInput")
    w1t_d = nc.dram_tensor("w1t", [128, KT_D, H], F32, kind="ExternalInput")
    w2t_d = nc.dram_tensor("w2t", [128, KT_H, K], F32, kind="ExternalInput")
    w2rows_d = nc.dram_tensor("w2rows", [K, H], F32, kind="ExternalInput")
    b1t_d = nc.dram_tensor("b1t", [128, KT_H], F32, kind="ExternalInput")
    b2row_d = nc.dram_tensor("b2row", [1, K], F32, kind="ExternalInput")
    selmat_d = nc.dram_tensor("selmat", [K, D], BF16, kind="ExternalInput")
    ident_d = nc.dram_tensor("ident", [128, 128], F32, kind="ExternalInput")
    identb_d = nc.dram_tensor("identb", [128, 128], BF16, kind="ExternalInput")
    scal_d = nc.dram_tensor("scal", [1, 4], F32, kind="ExternalInput")
    mats_d = nc.dram_tensor("mats", [K, 128, KT_D, D], BF16, kind="ExternalInput")
    iota16_d = nc.dram_tensor("iota16", [16, BT // 16], F32, kind="ExternalInput")
    positer_d = nc.dram_tensor("positer", [16, CW // 16], F32, kind="ExternalInput")
    outc_d = nc.dram_tensor("outc", [128, KT_D, CW], F32, kind="ExternalOutput")
    idxout_d = nc.dram_tensor("idxout", [16, CW // 16], F32, kind="ExternalOutput")
    if debug_outs:
        dbg_gated_d = nc.dram_tensor("dbg_gated", [128, NB, K], F32, kind="ExternalOutput")
        dbg_u2_d = nc.dram_tensor("dbg_u2", [128, NB, K], F32, kind="ExternalOutput")

    cc_in = nc.dram_tensor("cc_in", [128], F32)
    cc_out = nc.dram_tensor("cc_out", [128], F32, addr_space="Shared")

    with tile.TileContext(nc) as tc:
        with (
            tc.tile_pool(name="persist", bufs=1) as pp,
            tc.tile_pool(name="work", bufs=2) as wp,
            tc.tile_pool(name="rmats", bufs=2) as mp,
            tc.tile_pool(name="psP", bufs=1, space="PSUM") as psP,
        ):
            # ----- constants (once) -----
            w1t = pp.tile([128, KT_D, H], F32, tag="w1t")
            nc.sync.dma_start(w1t[:], w1t_d[:])
            w2t = pp.tile([128, KT_H, K], F32, tag="w2t")
            nc.sync.dma_start(w2t[:], w2t_d[:])
            b1t = pp.tile([128, KT_H], F32, tag="b1t")
            nc.sync.dma_start(b1t[:], b1t_d[:])
            b2r = pp.tile([1, K], F32, tag="b2r")
            nc.sync.dma_start(b2r[:], b2row_d[:])
            sel = pp.tile([K, D], BF16, tag="sel")
            nc.sync.dma_start(sel[:], selmat_d[:])
            ident = pp.tile([128, 128], F32, tag="ident")
            nc.sync.dma_start(ident[:], ident_d[:])
            identb = pp.tile([128, 128], BF16, tag="identb")
            nc.sync.dma_start(identb[:], identb_d[:])
            scal = pp.tile([1, 4], F32, tag="scal")
            nc.sync.dma_start(scal[:], scal_d[:])
            w1tr = pp.tile([128, KT_D, H], F32R, tag="w1tr")
            nc.vector.tensor_copy(w1tr[:], w1t[:])
            w2bc = []
            for k in range(K):
                t = pp.tile([1, H], F32, tag=f"w2row{k}", name=f"w2row{k}")
                nc.sync.dma_start(t[:], w2rows_d[k:k + 1, :])
                tb = pp.tile([128, H], F32, tag=f"w2bc{k}", name=f"w2bc{k}")
                nc.gpsimd.partition_broadcast(tb[:], t[:])
                w2bc.append(tb)
            b2bc = pp.tile([128, K], F32, tag="b2bc")
            nc.gpsimd.partition_broadcast(b2bc[:], b2r[:])

            # ----- weight-only precompute (once): G = W1 @ W1^T, G2 pairs -----
            G = pp.tile([128, KT_H, H], F32, tag="G")
            for ms in range(KT_H):
                ps = psP.tile([128, H], F32, tag="small", bufs=2, name=f"gps{ms}")
                for kt in range(KT_D):
                    nc.tensor.matmul(ps[:], w1tr[:, kt, ms * 128:(ms + 1) * 128],
                                     w1tr[:, kt, :], start=(kt == 0),
                                     stop=(kt == KT_D - 1))
                nc.vector.tensor_copy(G[:, ms, :], ps[:])
            G2p = []
            for p in range(K // 2):
                g2 = pp.tile([128, KT_H, 2 * H], BF16, tag=f"g2_{p}",
                             name=f"g2_{p}")
                for half, k in ((0, 2 * p), (1, 2 * p + 1)):
                    hs = slice(half * H, (half + 1) * H)
                    for kt in range(KT_H):
                        nc.vector.tensor_scalar(g2[:, kt, hs], G[:, kt, :],
                                                w2t[:, kt, k:k + 1], None, ALU.mult)
                        nc.vector.tensor_tensor(g2[:, kt, hs], g2[:, kt, hs],
                                                w2bc[k][:], ALU.mult)
                G2p.append(g2)

            for rep in range(repeat):
                # ----- x^T load: one 4MB DMA -----
                xTall = pp.tile([128, KT_D, BT], F32, tag="xTall",
                                name=f"xT_{rep}")
                nc.sync.dma_start(xTall[:], featT_d[:])

                if phase == "null":
                    break

                # mats prefetch: 3-deep ring, spread across issuers
                engs = [nc.scalar, nc.gpsimd, nc.sync]
                matj = []
                for j in range(K):
                    t = mp.tile([128, KT_D, D], BF16, tag="matj",
                                name=f"matj{j}_{rep}")
                    engs[j % 3].dma_start(t[:], mats_d[j])
                    matj.append(t)

                # ----- h = relu(W1 @ x^T + b1) in f32r; maskT = (h > 0) -----
                relu = pp.tile([128, KT_H, BT], F32, tag="relu", name=f"relu_{rep}")
                maskT = pp.tile([128, KT_H, BT], BF16, tag="maskT",
                                name=f"maskT_{rep}")
                for ns in range(NS):
                    nsl = slice(ns * 512, (ns + 1) * 512)
                    hacc = [psP.tile([128, 512], F32, tag=("B1", "B2")[ms],
                                     bufs=(2, 1)[ms], name=f"h{ms}_{ns}_{rep}")
                            for ms in range(KT_H)]
                    for kt in range(KT_D):
                        xr = wp.tile([128, 512], F32R, tag="xr",
                                     name=f"xr{kt}_{ns}_{rep}")
                        nc.scalar.copy(xr[:], xTall[:, kt, nsl])
                        for ms in range(KT_H):
                            nc.tensor.matmul(
                                hacc[ms][:], w1tr[:, kt, ms * 128:(ms + 1) * 128],
                                xr[:], start=(kt == 0), stop=(kt == KT_D - 1))
                    for ms in range(KT_H):
                        nc.scalar.activation(relu[:, ms, nsl], hacc[ms][:],
                                             ACTF.Relu, bias=b1t[:, ms:ms + 1],
                                             scale=1.0)
                        nc.vector.tensor_scalar(maskT[:, ms, nsl],
                                                relu[:, ms, nsl],
                                                0.0, None, ALU.is_gt)

                # maskB via PE transposes (bf16)
                maskB = pp.tile([128, NB, H], BF16, tag="maskB",
                                name=f"maskB_{rep}")
                for kt in range(KT_H):
                    for bt in range(NB):
                        tps = psP.tile([128, 128], BF16, tag="small", bufs=2,
                                       name=f"tps{kt}_{bt}_{rep}")
                        nc.tensor.transpose(tps[:], maskT[:, kt, bt * 128:(bt + 1) * 128],
                                            identb[:])
                        nc.vector.tensor_copy(maskB[:, bt, kt * 128:(kt + 1) * 128],
                                              tps[:])

                # ----- coeffs_B (fp32), u2_B (bf16 + fused mask-dot) -----
                coeffsB = pp.tile([128, NB, K], F32, tag="coeffsB",
                                  name=f"coeffsB_{rep}")
                u2 = pp.tile([128, NB, K], F32, tag="u2", name=f"u2_{rep}")
                for bt in range(NB):
                    bsl = slice(bt * 128, (bt + 1) * 128)
                    cps = psP.tile([128, K], F32, tag="small", bufs=2,
                                   name=f"cps{bt}_{rep}")
                    for kt in range(KT_H):
                        nc.tensor.matmul(cps[:], relu[:, kt, bsl], w2t[:, kt, :],
                                         start=(kt == 0), stop=(kt == KT_H - 1))
                    nc.vector.tensor_tensor(coeffsB[:, bt, :], cps[:], b2bc[:],
                                            ALU.add)
                    for pg in range(2):
                        xk_a = psP.tile([128, 2 * H], F32, tag="B1", bufs=2,
                                        name=f"xka_{bt}_{pg}_{rep}")
                        xk_b = psP.tile([128, 2 * H], F32, tag="B2", bufs=1,
                                        name=f"xkb_{bt}_{pg}_{rep}")
                        for kt in range(KT_H):
                            nc.tensor.matmul(xk_a[:], maskT[:, kt, bsl],
                                             G2p[2 * pg][:, kt, :],
                                             start=(kt == 0), stop=(kt == KT_H - 1))
                            nc.tensor.matmul(xk_b[:], maskT[:, kt, bsl],
                                             G2p[2 * pg + 1][:, kt, :],
                                             start=(kt == 0), stop=(kt == KT_H - 1))
                        for half_t, xk in ((0, xk_a), (1, xk_b)):
                            scr = wp.tile([128, 2, H], F32, tag="ttr_scratch",
                                          name=f"scr{bt}_{pg}_{half_t}_{rep}")
                            for half in range(2):
                                nc.vector.tensor_tensor(
                                    scr[:, half, :],
                                    xk[:, half * H:(half + 1) * H],
                                    maskB[:, bt, :], ALU.mult)
                            k0 = 4 * pg + 2 * half_t
                            nc.vector.tensor_reduce(
                                u2[:, bt, k0:k0 + 2], scr[:], AX.X, ALU.add)

                # ----- AllReduce max -----
                lmax = wp.tile([128, 1], F32, tag="lmax", name=f"lmax_{rep}")
                nc.vector.tensor_reduce(lmax[:], u2[:], AX.XY, ALU.max)
                nc.gpsimd.dma_start(cc_in[:], lmax[:])
                nc.gpsimd.collective_compute(
                    "AllReduce", ALU.max, replica_groups=[list(range(N_CORES))],
                    ins=[cc_in[:]], outs=[cc_out[:]])
                gmax_col = wp.tile([1, 128], F32, tag="gmax_col", name=f"gmax_{rep}")
                nc.gpsimd.dma_start(gmax_col[:], cc_out[:])
                m2 = wp.tile([1, 1], F32, tag="m2", name=f"m2_{rep}")
                nc.vector.tensor_reduce(m2[:], gmax_col[:], AX.X, ALU.max)
                sq = wp.tile([1, 1], F32, tag="sqm2", name=f"sq_{rep}")
                nc.scalar.activation(sq[:], m2[:], ACTF.Sqrt)
                rs = wp.tile([1, 1], F32, tag="rsq", name=f"rs_{rep}")
                nc.vector.reciprocal(rs[:], sq[:])
                s11 = wp.tile([1, 1], F32, tag="s11", name=f"s11_{rep}")
                nc.vector.tensor_tensor(s11[:], rs[:], scal[:, 1:2], ALU.mult)
                sbc = wp.tile([128, 1], F32, tag="sbc", name=f"sbc_{rep}")
                nc.gpsimd.partition_broadcast(sbc[:], s11[:])
                basebc = wp.tile([128, 1], F32, tag="basebc", name=f"basebc_{rep}")
                nc.gpsimd.partition_broadcast(basebc[:], scal[:, 0:1])

                # ----- thresholds + gating -----
                unionK = pp.tile([128, NB], F32, tag="unionK", name=f"unionK_{rep}")
                gatedB = pp.tile([128, NB, K], F32, tag="gatedB", name=f"gatedB_{rep}")
                for bt in range(NB):
                    sqr = wp.tile([128, K], F32, tag="sqr", name=f"sqr{bt}_{rep}")
                    nc.scalar.activation(sqr[:], u2[:, bt, :], ACTF.Sqrt)
                    th = wp.tile([128, K], F32, tag="th", name=f"th{bt}_{rep}")
                    nc.scalar.activation(th[:], sqr[:], ACTF.Identity,
                                         bias=basebc[:], scale=sbc[:])
                    absc = wp.tile([128, K], F32, tag="absc", name=f"absc{bt}_{rep}")
                    nc.scalar.activation(absc[:], coeffsB[:, bt, :], ACTF.Abs)
                    keep = wp.tile([128, K], F32, tag="keep", name=f"keep{bt}_{rep}")
                    nc.vector.tensor_tensor(keep[:], absc[:], th[:], ALU.is_ge)
                    nc.vector.tensor_tensor(gatedB[:, bt, :], coeffsB[:, bt, :],
                                            keep[:], ALU.mult)
                    nc.vector.tensor_reduce(unionK[:, bt:bt + 1], keep[:], AX.X, ALU.max)
                if debug_outs and rep == repeat - 1:
                    nc.sync.dma_start(dbg_gated_d[:], gatedB[:])
                    nc.sync.dma_start(dbg_u2_d[:], u2[:])

                gatedT = pp.tile([K, BT], F32, tag="gatedT", name=f"gatedT_{rep}")
                for bt in range(NB):
                    tps = psP.tile([K, 128], F32, tag="small", bufs=2,
                                   name=f"tpsg{bt}_{rep}")
                    nc.tensor.transpose(tps[:], gatedB[:, bt, :], ident[:])
                    nc.vector.tensor_copy(gatedT[:, bt * 128:(bt + 1) * 128], tps[:])
                gatedTb = pp.tile([K, BT], BF16, tag="gatedTb", name=f"gatedTb_{rep}")
                nc.vector.tensor_copy(gatedTb[:], gatedT[:])

                if upto == "gating":
                    continue
                # ----- combination loop -----
                # j=0 runs DENSE with the gate folded into the bf16 moving
                # operand; j=1..7 run on the compacted active columns (CW).
                WF = CW // 16

                # gb0 = gated[:, 0] broadcast to all 128 partitions
                gb0 = wp.tile([128, BT], F32, tag="gbsb", bufs=1, name=f"gb0_{rep}")
                for ns in range(NS):
                    gps_ = psP.tile([128, 512], F32, tag="small", bufs=2,
                                    name=f"gbp0_{ns}_{rep}")
                    nc.tensor.matmul(gps_[:], sel[:, 0:128],
                                     gatedTb[:, ns * 512:(ns + 1) * 512],
                                     start=True, stop=True)
                    nc.scalar.copy(gb0[:, ns * 512:(ns + 1) * 512], gps_[:])
                # rcur = xT * gb0 (bf16): gate applied on the moving side
                rcur = wp.tile([128, KT_D, BT], BF16, tag="rmoving", bufs=1,
                               name=f"r0_{rep}")
                for kt in range(KT_D):
                    for ns in range(NS):
                        nsl = slice(ns * 512, (ns + 1) * 512)
                        nc.vector.tensor_tensor(rcur[:, kt, nsl],
                                                xTall[:, kt, nsl],
                                                gb0[:, nsl], ALU.mult)
                # j=0 dense pass: xT += matj0^T @ rcur
                for m in range(KT_D):
                    for ns in range(NS):
                        nsl = slice(ns * 512, (ns + 1) * 512)
                        ps = psP.tile([128, 512], F32, tag="A", bufs=3,
                                      name=f"lps0_{m}_{ns}_{rep}")
                        for kt in range(KT_D):
                            nc.tensor.matmul(
                                ps[:], matj[0][:, kt, m * 128:(m + 1) * 128],
                                rcur[:, kt, nsl],
                                start=(kt == 0), stop=(kt == KT_D - 1))
                        nc.vector.tensor_tensor(xTall[:, m, nsl], xTall[:, m, nsl],
                                                ps[:], ALU.add)

                # --- index machinery (overlaps the j=0 matmuls above) ---
                iota16 = pp.tile([16, BT // 16], F32, tag="iota16", name=f"iota16_{rep}")
                nc.scalar.dma_start(iota16[:], iota16_d[:])
                positer = pp.tile([16, WF], F32, tag="positer", name=f"positer_{rep}")
                nc.scalar.dma_start(positer[:], positer_d[:])
                u16 = wp.tile([16, NB, 8], F32, tag="u16", name=f"u16_{rep}")
                for g in range(8):
                    nc.scalar.dma_start(u16[:, :, g:g + 1],
                                        unionK[g * 16:(g + 1) * 16, :])
                cand = wp.tile([16, BT // 16], F32, tag="cand", name=f"cand_{rep}")
                # u16 layout: [p16, g, bt] maps to b = bt*128 + g*16 + p16;
                # iota wrapped layout needs b = f*16 + p16 with f = bt*8 + g.
                # Reorder free dims via an AP view: (g, bt) -> (bt, g)
                u16flat = u16[:].rearrange("p b g -> p (b g)")
                nc.vector.tensor_tensor(cand[:], iota16[:], u16flat, ALU.mult)
                um1 = wp.tile([16, BT // 16], F32, tag="um1", name=f"um1_{rep}")
                nc.vector.tensor_scalar(um1[:], u16flat, 1.0, None, ALU.subtract)
                nc.vector.tensor_tensor(cand[:], cand[:], um1[:], ALU.add)
                idxf = wp.tile([16, WF], F32, tag="idxf", name=f"idxf_{rep}")
                cnt = wp.tile([1, 1], mybir.dt.uint32, tag="cnt", name=f"cnt_{rep}")
                nc.gpsimd.sparse_gather(idxf[:], cand[:], num_found=cnt[:])
                # mask junk tail: idx = idx if pos < cnt else -1
                cntf = wp.tile([1, 1], F32, tag="cntf", name=f"cntf_{rep}")
                nc.vector.tensor_copy(cntf[:], cnt[:])
                cntbc = wp.tile([16, 1], F32, tag="cntbc", name=f"cntbc_{rep}")
                nc.gpsimd.partition_broadcast(cntbc[:], cntf[:])
                posok = wp.tile([16, WF], F32, tag="posok", name=f"posok_{rep}")
                nc.vector.tensor_scalar(posok[:], positer[:], cntbc[:], None, ALU.is_lt)
                nc.vector.tensor_tensor(idxf[:], idxf[:], posok[:], ALU.mult)
                pm1 = wp.tile([16, WF], F32, tag="pm1", name=f"pm1_{rep}")
                nc.vector.tensor_scalar(pm1[:], posok[:], 1.0, None, ALU.subtract)
                nc.vector.tensor_tensor(idxf[:], idxf[:], pm1[:], ALU.add)
                nc.scalar.dma_start(idxout_d[:], idxf[:])
                idxf0 = wp.tile([16, WF], F32, tag="idxf0", name=f"idxf0_{rep}")
                nc.vector.tensor_scalar(idxf0[:], idxf[:], 0.0, None, ALU.max)
                idx16 = wp.tile([16, WF], I16, tag="idx16", name=f"idx16_{rep}")
                nc.vector.tensor_copy(idx16[:], idxf0[:])
                idxrep = wp.tile([128, WF], I16, tag="idxrep", name=f"idxrep_{rep}")
                for g in range(8):
                    nc.scalar.dma_start(idxrep[g * 16:(g + 1) * 16, :], idx16[:])

                # gatedTc: compact gated rows
                gatedT16 = pp.tile([16, BT], F32, tag="gatedT16", name=f"gatedT16_{rep}")
                nc.vector.memset(gatedT16[:], 0.0)
                nc.vector.tensor_copy(gatedT16[0:K, :], gatedT[:])
                gatedTc = pp.tile([16, CW], F32, tag="gatedTc", name=f"gatedTc_{rep}")
                nc.gpsimd.ap_gather(gatedTc[:], gatedT16[:], idx16[:], channels=16,
                                    num_elems=BT, d=1, num_idxs=CW)
                gatedTcb = pp.tile([K, CW], BF16, tag="gatedTcb", name=f"gatedTcb_{rep}")
                nc.vector.tensor_copy(gatedTcb[:], gatedTc[0:K, :])

                # gather compacted x (after j=0 updates)
                xc = []
                for m in range(KT_D):
                    t = pp.tile([128, CW], F32, tag=f"xc{m}", name=f"xc{m}_{rep}")
                    nc.gpsimd.ap_gather(t[:], xTall[:, m, :], idxrep[:], channels=128,
                                        num_elems=BT, d=1, num_idxs=CW)
                    xc.append(t)

                # compact loop j=1..7 (software-pipelined: rc/gbc for j+1
                # are produced during step j right after each xc[m] update)
                def make_gbc(j):
                    gbc = wp.tile([128, CW], F32, tag="gbc", name=f"gbc{j}_{rep}")
                    gcp = psP.tile([128, CW], F32, tag="small", bufs=2,
                                   name=f"gcp{j}_{rep}")
                    nc.tensor.matmul(gcp[:], sel[:, j * 128:(j + 1) * 128],
                                     gatedTcb[:], start=True, stop=True)
                    nc.scalar.copy(gbc[:], gcp[:])
                    return gbc

                if upto == "full":
                    gbc = make_gbc(1)
                    rc = wp.tile([128, KT_D, CW], BF16, tag="rc", name=f"rc1_{rep}")
                    for m in range(KT_D):
                        nc.vector.tensor_tensor(rc[:, m, :], xc[m][:], gbc[:], ALU.mult)
                    for j in range(1, K):
                        gbcn = make_gbc(j + 1) if j + 1 < K else None
                        rcn = (wp.tile([128, KT_D, CW], BF16, tag="rc",
                                       name=f"rc{j+1}_{rep}") if j + 1 < K else None)
                        for m in range(KT_D):
                            ps = psP.tile([128, CW], F32, tag="A", bufs=3,
                                          name=f"clps{j}_{m}_{rep}")
                            for kt in range(KT_D):
                                nc.tensor.matmul(
                                    ps[:], matj[j][:, kt, m * 128:(m + 1) * 128],
                                    rc[:, kt, :],
                                    start=(kt == 0), stop=(kt == KT_D - 1))
                            nc.vector.tensor_tensor(xc[m][:], xc[m][:], ps[:], ALU.add)
                            if rcn is not None:
                                nc.vector.tensor_tensor(rcn[:, m, :], xc[m][:],
                                                        gbcn[:], ALU.mult)
                        rc = rcn

            for kt in range(KT_D):
                if phase != "null" and upto != "gating":
                    nc.sync.dma_start(outc_d[:, kt, :], xc[kt][:])

    nc.finalize()
    return nc


def make_inputs(features_shard_T, W1, b1, W2, b2, task_mats, base_threshold, beta,
                CW=CW_DEFAULT):
    import ml_dtypes
    BT = features_shard_T.shape[1]
    featT = np.ascontiguousarray(
        features_shard_T.reshape(KT_D, 128, BT).transpose(1, 0, 2))
    w1t = np.ascontiguousarray(W1.T.reshape(KT_D, 128, H).transpose(1, 0, 2))
    w2t = np.ascontiguousarray(W2.T.reshape(KT_H, 128, K).transpose(1, 0, 2))
    b1t = np.ascontiguousarray(b1.reshape(KT_H, 128).T)
    sel = np.zeros((K, D), np.float32)
    for j in range(K):
        sel[j, j * 128:(j + 1) * 128] = 1.0
    base_sp = np.log1p(np.exp(np.float32(base_threshold[0]))).astype(np.float32)
    rbeta = np.maximum(np.float32(beta[0]), 0).astype(np.float32)
    scal = np.array([[base_sp, base_sp * rbeta, 0, 0]], np.float32)
    mats = np.ascontiguousarray(
        task_mats.reshape(K, KT_D, 128, D).transpose(0, 2, 1, 3)).astype(ml_dtypes.bfloat16)
    iota16 = (np.arange(BT).reshape(BT // 16, 16).T).astype(np.float32)
    positer = (np.arange(CW).reshape(CW // 16, 16).T).astype(np.float32)
    return {
        "iota16": np.ascontiguousarray(iota16),
        "positer": np.ascontiguousarray(positer),
        "featT": featT.astype(np.float32),
        "w1t": w1t.astype(np.float32),
        "w2t": w2t.astype(np.float32),
        "w2rows": np.ascontiguousarray(W2).astype(np.float32),
        "b1t": b1t.astype(np.float32),
        "b2row": b2.reshape(1, K).astype(np.float32),
        "selmat": sel.astype(ml_dtypes.bfloat16),
        "ident": np.eye(128, dtype=np.float32),
        "identb": np.eye(128, dtype=np.float32).astype(ml_dtypes.bfloat16),
        "scal": scal,
        "mats": mats,
    }




_CACHE = {}


def _host_reference(features, W1, b1, W2, b2, task_mats, base_threshold, beta):
    """Pure-numpy fallback (only for pathological gating densities)."""
    f = features.astype(np.float64)
    h = f @ W1.T.astype(np.float64) + b1.astype(np.float64)
    relu_h = np.maximum(h, 0.0)
    coeffs = relu_h @ W2.T.astype(np.float64) + b2.astype(np.float64)
    mask = (h > 0).astype(np.float64)
    G = W1.astype(np.float64) @ W1.T.astype(np.float64)
    u2 = np.zeros((f.shape[0], K))
    for k in range(K):
        V = mask * W2[k].astype(np.float64)[None, :]
        u2[:, k] = np.sum((V @ G) * V, axis=1)
    unc = np.sqrt(u2)
    mx = unc.max()
    if mx > 0:
        unc = unc / mx
    base = np.log1p(np.exp(np.float64(base_threshold.reshape(-1)[0])))
    th = base * (1.0 + max(float(beta.reshape(-1)[0]), 0.0) * unc)
    gated = np.where(np.abs(coeffs) < th, 0.0, coeffs)
    x = f.copy()
    for j in range(K):
        x = x + gated[:, j:j + 1] * (x @ task_mats[j].astype(np.float64))
    return x.astype(np.float32)


def _union_counts(features, W1, b1, W2, b2, base_threshold, beta):
    """Approximate per-core union-active counts (sizing check only)."""
    f = features.astype(np.float32)
    h = f @ W1.T + b1
    coeffs = np.maximum(h, 0) @ W2.T + b2
    mask = (h > 0).astype(np.float32)
    G = (W1 @ W1.T).astype(np.float32)
    u2 = np.zeros((f.shape[0], K), np.float32)
    for k in range(K):
        V = mask * W2[k][None, :]
        u2[:, k] = np.sum((V @ G) * V, axis=1)
    unc = np.sqrt(u2)
    mx = unc.max()
    if mx > 0:
        unc = unc / mx
    base = np.log1p(np.exp(np.float32(base_threshold.reshape(-1)[0])))
    th = base * (1 + max(float(beta.reshape(-1)[0]), 0.0) * unc)
    active = (np.abs(coeffs) >= th).any(axis=1)
    BT = f.shape[0] // N_CORES
    return [int(active[c * BT:(c + 1) * BT].sum()) for c in range(N_CORES)]


def kernel(features, W1, b1, W2, b2, task_mats, proj_W, base_threshold, beta,
           **_unused):
    from concourse.bass_utils import run_bass_kernel_spmd

    features = np.asarray(features, dtype=np.float32)
    W1 = np.asarray(W1, np.float32)
    b1 = np.asarray(b1, np.float32)
    W2 = np.asarray(W2, np.float32)
    b2 = np.asarray(b2, np.float32)
    task_mats = np.asarray(task_mats, np.float32)
    base_threshold = np.asarray(base_threshold, np.float32)
    beta = np.asarray(beta, np.float32)
    B = features.shape[0]
    BT = B // N_CORES
    CW = CW_DEFAULT

    # sizing check: the compact loop holds up to CW active columns per core.
    # device/host gate decisions can differ by a couple of borderline samples,
    # so require a safety margin; otherwise fall back to exact host compute.
    counts = _union_counts(features, W1, b1, W2, b2, base_threshold, beta)
    x = None
    if max(counts) <= CW - 24 and B % N_CORES == 0:
        try:
            key = ("nc", BT, CW)
            if key not in _CACHE:
                _CACHE[key] = build(BT=BT, CW=CW)
            nc = _CACHE[key]
            in_maps = []
            for c in range(N_CORES):
                shard_T = np.ascontiguousarray(features[c * BT:(c + 1) * BT].T)
                in_maps.append(make_inputs(shard_T, W1, b1, W2, b2, task_mats,
                                           base_threshold, beta, CW=CW))
            res = run_bass_kernel_spmd(nc, in_maps, core_ids=list(range(N_CORES)))
            outs = []
            for c in range(N_CORES):
                r = res.results[c]
                idx1d = r["idxout"].T.reshape(-1)      # unwrap i = f*16 + p
                valid = idx1d >= 0
                idxs = idx1d[valid].astype(np.int64)
                xcT = r["outc"].transpose(1, 0, 2).reshape(D, CW)
                xfull = features[c * BT:(c + 1) * BT].copy()
                xfull[idxs, :] = xcT[:, valid].T
                outs.append(xfull)
            x = np.concatenate(outs, axis=0)
        except Exception as e:
            import traceback
            print(f"kernel: device path failed ({e!r}); falling back to host",
                  file=sys.stderr)
            traceback.print_exc()
            x = None
    if x is None:
        x = _host_reference(features, W1, b1, W2, b2, task_mats,
                            base_threshold, beta)

    proj_W = np.asarray(proj_W, np.float32)
    if not (proj_W.shape == (D, D) and
            np.array_equal(proj_W, np.eye(D, dtype=proj_W.dtype))):
        x = (x @ proj_W.T).astype(np.float32)
    return np.ascontiguousarray(x.astype(np.float32))
